# revision 2
# baseline (speedup 1.0000x reference)
"""Causal self-attention (B=2, S=2048, C=1024, H=16) on 8 TRN2 NeuronCores.

Sharding: tensor-parallel over heads — 2 heads per core. Each core computes
  qkv.T = w_c.T @ x.T          (its 384 qkv columns, transposed layout)
  scores.T = k @ q.T           (per head, [sk, sq] layout, causal-blocked)
  P.T = exp(scores.T / 8)      (no max-subtraction; scores ~ N(0,1))
  y_aug.T = [v | 1].T @ P.T    (row 64 = softmax denominators)
  y_norm.T = y.T / sums        (broadcast via gpsimd partition_broadcast)
  out_partial = y_norm @ w_proj_c   ([4096, 1024] partial over head dims)
Host sums the 8 partials and adds biases (b_attn is folded in on-device).

Matmuls run as float32r (PE full rate); transposes/accumulation stay fp32.
"""

import os
from contextlib import ExitStack

import numpy as np

import concourse.bass as bass
import concourse.tile as tile
from concourse import bacc, mybir
from concourse.bass_utils import run_bass_kernel_spmd
from concourse.masks import make_identity

F32 = mybir.dt.float32

N_HEAD = 16
N_EMBD = 1024
B = 2
S = 2048
C = N_EMBD
D = C // N_HEAD  # 64
N_CORES = 8
HPC = N_HEAD // N_CORES  # 2 heads per core

# matmul compute dtype: float32r = full-rate PE, reduced-precision multiplies
# (requires producer instructions to write f32r-rounded tiles).
MM_DT = mybir.dt.float32r if os.environ.get("ATTN_MM_DT", "f32r") == "f32r" else F32

LAST_EXEC_NS = None  # set by kernel() when profiling info is available


def _mm(ap):
    return ap


def build_nc(s_per_batch=S, n_batch=B):
    """Build the single-core SPMD program. Returns the Bass object."""
    sq = n_batch * s_per_batch          # total rows (flattened B*S)
    n_j = sq // 512                     # 512-wide sq chunks over all rows
    n_j4 = s_per_batch // 512           # 512-wide sq chunks per batch
    n_sk = s_per_batch // 128           # 128-tall sk tiles per batch
    w_cols = 3 * HPC * D                # 384

    nc = bacc.Bacc("TRN2", target_bir_lowering=False, debug=False)

    x = nc.dram_tensor("x", [sq, C], F32, kind="ExternalInput").ap()
    w_qkv = nc.dram_tensor("w_qkv", [C, w_cols], MM_DT, kind="ExternalInput").ap()
    b_qkv = nc.dram_tensor("b_qkv", [w_cols, 1], F32, kind="ExternalInput").ap()
    w_proj = nc.dram_tensor("w_proj", [HPC * D, C], MM_DT, kind="ExternalInput").ap()
    out = nc.dram_tensor("out", [sq, C], F32, kind="ExternalOutput").ap()
    # DRAM scratch for the softmax-denominator partition broadcast
    n_sums = n_batch * n_j4 * HPC
    sums_dram = nc.dram_tensor("sums_scratch", [n_sums, 512], F32).ap()

    with tile.TileContext(nc) as tc, ExitStack() as ctx:
        persist = ctx.enter_context(tc.tile_pool(name="persist", bufs=1))
        xrow_pool = ctx.enter_context(tc.tile_pool(name="xrow", bufs=6))
        xt_pool = ctx.enter_context(tc.tile_pool(name="xt", bufs=10))
        pt_pool = ctx.enter_context(tc.tile_pool(name="pt", bufs=4))
        small_pool = ctx.enter_context(tc.tile_pool(name="small", bufs=4))
        outsb_pool = ctx.enter_context(tc.tile_pool(name="outsb", bufs=4))

        phase1_ctx = ExitStack()
        ps_tr = phase1_ctx.enter_context(
            tc.tile_pool(name="ps_tr", bufs=2, space="PSUM"))
        ps_qkv = phase1_ctx.enter_context(
            tc.tile_pool(name="ps_qkv", bufs=3, space="PSUM"))

        # --- persistent sbuf tensors ---
        identity = persist.tile([128, 128], F32, tag="identity")
        make_identity(nc, identity)

        w_sb = []
        for k in range(C // 128):
            wt = persist.tile([128, w_cols], MM_DT, tag=f"w{k}", name=f"w_sb{k}")
            nc.sync.dma_start(out=wt, in_=w_qkv[128 * k:128 * (k + 1), :])
            w_sb.append(wt)

        battn_sb = persist.tile([128, 3], F32, tag="battn")
        for m in range(3):
            nc.sync.dma_start(
                out=battn_sb[:, m:m + 1],
                in_=b_qkv[128 * m:128 * (m + 1), :],
            )

        wproj_sb = persist.tile([128, C], MM_DT, tag="wproj")
        nc.sync.dma_start(out=wproj_sb, in_=w_proj)

        # qkv.T tiles: [0]=q.T, [1]=k.T, [2]=v.T ; rows 0-63 head0, 64-127 head1
        qkvT = [
            persist.tile([128, sq], MM_DT if m < 2 else F32,
                         tag=f"qkvT{m}", name=f"qkvT{m}")
            for m in range(3)
        ]
        # v in natural layout, augmented with a ones column: per head,
        # n_batch*n_sk blocks of [128 sk, 65] packed along the free dim.
        n_blk = n_batch * n_sk
        v_sb = [
            persist.tile([128, 65 * n_blk], MM_DT, tag=f"v{h}", name=f"v_sb{h}")
            for h in range(HPC)
        ]
        # normalized y.T: rows = 2 heads x 64 dims, cols = all sq
        ynorm = persist.tile([128, sq], MM_DT, tag="ynorm")

        # ---------------- phase 1: x.T and qkv.T ----------------
        for j in range(n_j):
            xrows = []
            for p in range(4):
                xr = xrow_pool.tile([128, C], F32, name=f"xr_{j}_{p}", tag="xr")
                nc.sync.dma_start(
                    out=xr, in_=x[512 * j + 128 * p:512 * j + 128 * (p + 1), :]
                )
                xrows.append(xr)
            xts = []
            for k in range(C // 128):
                tp = ps_tr.tile([128, 512], F32, name=f"tp_{j}_{k}", tag="tp")
                for p in range(4):
                    nc.tensor.transpose(
                        tp[:, 128 * p:128 * (p + 1)],
                        xrows[p][:, 128 * k:128 * (k + 1)],
                        identity,
                    )
                xt = xt_pool.tile([128, 512], MM_DT, name=f"xt_{j}_{k}", tag="xt")
                nc.vector.tensor_copy(xt, tp)
                xts.append(xt)
            for m in range(3):
                qp = ps_qkv.tile([128, 512], F32, name=f"qp_{j}_{m}", tag="qp")
                for k in range(C // 128):
                    nc.tensor.matmul(
                        qp,
                        _mm(w_sb[k][:, 128 * m:128 * (m + 1)]),
                        _mm(xts[k]),
                        start=(k == 0),
                        stop=(k == C // 128 - 1),
                    )
                nc.vector.tensor_scalar_add(
                    qkvT[m][:, 512 * j:512 * (j + 1)], qp, battn_sb[:, m:m + 1]
                )

        # ---------------- phase 1.5: v natural layout ----------------
        for g in range(n_blk // 4):
            tp = ps_tr.tile([128, 512], F32, name=f"vtp_{g}", tag="tp")
            for p in range(4):
                blk = 4 * g + p
                nc.tensor.transpose(
                    tp[:, 128 * p:128 * (p + 1)],
                    qkvT[2][:, 128 * blk:128 * (blk + 1)],
                    identity,
                )
            for h in range(HPC):
                src = tp.rearrange("a (n c) -> a n c", c=128)[:, :, 64 * h:64 * h + 64]
                dst = (
                    v_sb[h][:, 65 * 4 * g:65 * 4 * (g + 1)]
                    .rearrange("a (n c) -> a n c", c=65)[:, :, 0:64]
                )
                nc.vector.tensor_copy(dst, src)
        ones_stage = persist.tile([128, n_blk], F32, tag="ones_stage")
        nc.vector.memset(ones_stage, 1.0)
        for h in range(HPC):
            ones_col = (v_sb[h].rearrange("a (n c) -> a n c", c=65)[:, :, 64:65]
                        .squeeze(2))
            nc.vector.tensor_copy(ones_col, ones_stage)

        # ---------------- phase 2: attention ----------------
        phase1_ctx.close()
        phase2_ctx = ExitStack()
        ps_s = phase2_ctx.enter_context(
            tc.tile_pool(name="ps_s", bufs=2, space="PSUM"))
        ps_y = phase2_ctx.enter_context(
            tc.tile_pool(name="ps_y", bufs=2, space="PSUM"))

        for b in range(n_batch):
            for j4 in range(n_j4):
                ni = 4 * j4 + 4  # causal: sk tiles 0..ni-1
                col0 = s_per_batch * b + 512 * j4  # global sq col of this chunk
                yps = [
                    ps_y.tile([128, 512], F32, name=f"y_{b}_{j4}_{h}", tag=f"y{h}")
                    for h in range(HPC)
                ]
                for i in range(ni):
                    sp = ps_s.tile([128, 1024], F32, name=f"s_{b}_{j4}_{i}", tag="s")
                    for h in range(HPC):
                        nc.tensor.matmul(
                            sp[:, 512 * h:512 * (h + 1)],
                            _mm(qkvT[1][64 * h:64 * (h + 1),
                                        s_per_batch * b + 128 * i:
                                        s_per_batch * b + 128 * (i + 1)]),
                            _mm(qkvT[0][64 * h:64 * (h + 1), col0:col0 + 512]),
                            start=True,
                            stop=True,
                        )
                    pt = pt_pool.tile([128, 1024], MM_DT, name=f"pt_{b}_{j4}_{i}",
                                      tag="ptt")
                    nc.scalar.activation(
                        pt, sp, mybir.ActivationFunctionType.Exp, scale=0.125
                    )
                    if 128 * i + 127 > 512 * j4:  # tile straddles the diagonal
                        for h in range(HPC):
                            sl = pt[:, 512 * h:512 * (h + 1)]
                            # keep where sq >= sk: (512*j4 + y) - (128*i + p) >= 0
                            nc.gpsimd.affine_select(
                                out=sl,
                                in_=sl,
                                pattern=[[1, 512]],
                                channel_multiplier=-1,
                                base=512 * j4 - 128 * i,
                                compare_op=mybir.AluOpType.is_ge,
                                fill=0.0,
                            )
                    for h in range(HPC):
                        blk = n_sk * b + i
                        nc.tensor.matmul(
                            yps[h][0:65, :],
                            _mm(v_sb[h][:, 65 * blk:65 * (blk + 1)]),
                            _mm(pt[:, 512 * h:512 * (h + 1)]),
                            start=(i == 0),
                            stop=(i == ni - 1),
                        )
                # softmax normalization, per head
                for h in range(HPC):
                    sums = small_pool.tile([128, 512], F32,
                                           name=f"sums_{b}_{j4}_{h}", tag="sums")
                    nc.vector.tensor_copy(sums[64:65, :], yps[h][64:65, :])
                    idx = (b * n_j4 + j4) * HPC + h
                    nc.sync.dma_start(
                        out=sums_dram[idx:idx + 1, :], in_=sums[64:65, :]
                    )
                    bcast = small_pool.tile([64, 512], F32,
                                            name=f"bc_{b}_{j4}_{h}", tag="bc")
                    row = sums_dram[idx:idx + 1, :]
                    row_b = bass.AP(
                        tensor=row.tensor, offset=row.offset,
                        ap=[[0, 64]] + list(row.ap[1:]),
                    )
                    nc.sync.dma_start(out=bcast, in_=row_b)
                    recip = small_pool.tile([64, 512], F32,
                                            name=f"rc_{b}_{j4}_{h}", tag="rc")
                    nc.vector.reciprocal(recip, bcast)
                    if h == 0:
                        nc.vector.tensor_mul(
                            ynorm[0:64, col0:col0 + 512], yps[h][0:64, :], recip
                        )
                    else:
                        ytmp = small_pool.tile([64, 512], MM_DT,
                                               name=f"yt_{b}_{j4}", tag="yt")
                        nc.vector.tensor_mul(ytmp, yps[h][0:64, :], recip)
                        nc.sync.dma_start(
                            out=ynorm[64:128, col0:col0 + 512], in_=ytmp
                        )

        # ---------------- phase 3: projection ----------------
        phase2_ctx.close()
        ps_pr = ctx.enter_context(tc.tile_pool(name="ps_pr", bufs=4, space="PSUM"))
        for t in range(sq // 128):
            for n in range(C // 512):
                pp = ps_pr.tile([128, 512], F32, name=f"pp_{t}_{n}", tag="pp")
                nc.tensor.matmul(
                    pp,
                    _mm(ynorm[:, 128 * t:128 * (t + 1)]),
                    _mm(wproj_sb[:, 512 * n:512 * (n + 1)]),
                    start=True,
                    stop=True,
                )
                ob = outsb_pool.tile([128, 512], F32, name=f"ob_{t}_{n}", tag="ob")
                nc.vector.tensor_copy(ob, pp)
                nc.sync.dma_start(
                    out=out[128 * t:128 * (t + 1), 512 * n:512 * (n + 1)], in_=ob
                )

    nc.compile()
    return nc


def shard_inputs(x, w_attn, b_attn, w_proj, s_per_batch=S, n_batch=B):
    """Build the 8 per-core input maps."""
    xf = np.ascontiguousarray(
        np.asarray(x, dtype=np.float32).reshape(-1, C)[: n_batch * s_per_batch]
    )
    w_attn = np.asarray(w_attn, dtype=np.float32)
    b_attn = np.asarray(b_attn, dtype=np.float32)
    w_proj = np.asarray(w_proj, dtype=np.float32)
    in_maps = []
    for c in range(N_CORES):
        heads = [HPC * c + h for h in range(HPC)]
        cols = []
        for part in range(3):  # q, k, v
            for h in heads:
                cols.append(np.arange(part * C + D * h, part * C + D * (h + 1)))
        cols = np.concatenate(cols)
        w_qkv_c = np.ascontiguousarray(w_attn[:, cols])
        b_qkv_c = np.ascontiguousarray(b_attn[cols].reshape(-1, 1))
        w_proj_c = np.ascontiguousarray(w_proj[D * heads[0]:D * (heads[-1] + 1), :])
        in_maps.append(
            {"x": xf, "w_qkv": w_qkv_c, "b_qkv": b_qkv_c, "w_proj": w_proj_c}
        )
    return in_maps


def kernel(x, w_attn, b_attn, w_proj, b_proj):
    global LAST_EXEC_NS, LAST_RES
    x = np.asarray(x, dtype=np.float32)
    Bv, Sv, Cv = x.shape
    assert (Bv, Sv, Cv) == (B, S, C), (Bv, Sv, Cv)
    nc = build_nc()
    in_maps = shard_inputs(x, w_attn, b_attn, w_proj)
    extra = {}
    if os.environ.get("ATTN_TMPDIR"):
        extra["tmpdir"] = os.environ["ATTN_TMPDIR"]
    res = run_bass_kernel_spmd(nc, in_maps, list(range(N_CORES)), **extra)
    LAST_EXEC_NS = res.exec_time_ns
    LAST_RES = res
    acc = np.zeros((B * S, C), dtype=np.float32)
    for r in res.results:
        acc += np.asarray(r["out"], dtype=np.float32)
    acc += np.asarray(b_proj, dtype=np.float32)[None, :]
    return acc.reshape(B, S, C)



# revision 29
# speedup vs baseline: 1.0748x; 1.0748x over previous
"""Causal self-attention (B=2, S=2048, C=1024, H=16) on 8 TRN2 NeuronCores.

Sharding: tensor-parallel over heads - 2 heads per core. Each core computes
  qkv.T = w_c.T @ x.T        (its 384 qkv columns, bf16, x transposed on PE
                              via regular matmuls against identity)
  scores.T = k @ q.T         (per head, [sk, sq] bf16, causal pair-blocked)
  P.T = exp(scores.T / 8)    (fp8 e4m3; no max-subtraction; scores ~ N(0,1))
  y_aug.T = [v | 1].T @ P.T  (fp8 DoubleRow, row 64 = softmax denominators)
  y_norm.T = y.T * (1/sums)  (reciprocal on [1,512], broadcast via PE matmul)
  out_partial = y_norm @ w_proj_c  ([4096, 1024] bf16 partial over head dims)
Host sums the 8 partials (upcast) and adds biases (b_attn folded on-device).

Emission interleaves qkv-prep / attention / projection so the scalar-engine
exp stream overlaps PE work and DMA stays busy throughout.
"""

import os
from collections import deque
from contextlib import ExitStack

import numpy as np

import concourse.bass as bass
import concourse.tile as tile
from concourse import bacc, mybir
from concourse.bass_utils import run_bass_kernel_spmd
from concourse.masks import make_identity

F32 = mybir.dt.float32
F32R = mybir.dt.float32r
BF16 = mybir.dt.bfloat16
FP8 = mybir.dt.float8e4
FP8E5 = mybir.dt.float8e5

N_HEAD = 16
N_EMBD = 1024
B = 2
S = 2048
C = N_EMBD
D = C // N_HEAD  # 64
N_CORES = 8
HPC = N_HEAD // N_CORES  # 2 heads per core

SQ = B * S            # 4096 flattened rows
N_CH = S // 512       # 512-row chunks per batch
N_SK = S // 128       # 128-row sk tiles per batch
W_COLS = 3 * HPC * D  # 384
VSTRIDE = 80          # fp8 v block stride: 64 v + 1 ones + 15 pad (16B align)

LAST_EXEC_NS = None
LAST_RES = None

USE_FP8 = os.environ.get("ATTN_FP8", "0") == "1"
USE_DR = USE_FP8 and os.environ.get("ATTN_DR", "1") == "1"
# P in e5m2: huge dynamic range (max 57344, subnormals to 2^-16) so plain
# exp(s/8) fits with no bias shift; V in e4m3 for its 3-bit mantissa.
PT_DT = FP8E5 if USE_FP8 else BF16
V_DT = FP8 if USE_FP8 else BF16


def build_nc():
    nc = bacc.Bacc("TRN2", target_bir_lowering=False, debug=False)

    x = nc.dram_tensor("x", [SQ, C], BF16, kind="ExternalInput").ap()
    w_qkv = nc.dram_tensor("w_qkv", [C, W_COLS], BF16, kind="ExternalInput").ap()
    b_qkv = nc.dram_tensor("b_qkv", [W_COLS, 1], F32, kind="ExternalInput").ap()
    w_proj = nc.dram_tensor("w_proj", [HPC * D, C], BF16, kind="ExternalInput").ap()
    out = nc.dram_tensor("out", [SQ, C], BF16, kind="ExternalOutput").ap()
    DEBUG = os.environ.get("ATTN_DEBUG", "0") == "1"
    if DEBUG:
        dbg_v = nc.dram_tensor(
            "dbg_v", [128, VSTRIDE * B * N_SK], PT_DT, kind="ExternalOutput"
        ).ap()
        dbg_pt = nc.dram_tensor(
            "dbg_pt", [128, 1024], PT_DT, kind="ExternalOutput"
        ).ap()
        dbg_rc = nc.dram_tensor(
            "dbg_rc", [16, 512], F32, kind="ExternalOutput"
        ).ap()

    with tile.TileContext(nc) as tc, ExitStack() as ctx:
        persist = ctx.enter_context(tc.tile_pool(name="persist", bufs=1))
        xrow_pool = ctx.enter_context(tc.tile_pool(name="xrow", bufs=8))
        xt_pool = ctx.enter_context(tc.tile_pool(name="xt", bufs=16))
        pt_pool = ctx.enter_context(tc.tile_pool(name="pt", bufs=4))
        small_pool = ctx.enter_context(tc.tile_pool(name="small", bufs=4))
        ob_pool = ctx.enter_context(tc.tile_pool(name="ob", bufs=4))
        ps_sp = ctx.enter_context(tc.tile_pool(name="ps_sp", bufs=2, space="PSUM"))
        ps_y = ctx.enter_context(tc.tile_pool(name="ps_y", bufs=2, space="PSUM"))
        ps_u = ctx.enter_context(tc.tile_pool(name="ps_u", bufs=2, space="PSUM"))

        # ---- persistent sbuf ----
        identity = persist.tile([128, 128], BF16, tag="identity")
        make_identity(nc, identity)

        w_sb = []
        for k in range(C // 128):
            wt = persist.tile([128, W_COLS], BF16, tag=f"w{k}", name=f"w_sb{k}")
            nc.sync.dma_start(out=wt, in_=w_qkv[128 * k:128 * (k + 1), :])
            w_sb.append(wt)

        battn_sb = persist.tile([128, 3], F32, tag="battn")
        for m in range(3):
            nc.sync.dma_start(
                out=battn_sb[:, m:m + 1], in_=b_qkv[128 * m:128 * (m + 1), :]
            )

        wproj_sb = persist.tile([128, C], BF16, tag="wproj")
        nc.sync.dma_start(out=wproj_sb, in_=w_proj)

        expbias = persist.tile([128, 1], F32, tag="expbias")
        nc.vector.memset(expbias, 0.0)

        # qkv.T: [0]=q.T, [1]=k.T (bf16), [2]=v.T (bf16, transposed back later)
        qkvT = [
            persist.tile([128, SQ], BF16, tag=f"qkvT{m}", name=f"qkvT{m}")
            for m in range(3)
        ]
        # v in natural layout fp8, augmented ones column at offset 64 of each
        # VSTRIDE block (whole tile memset to 1; v blocks overwrite cols 0-63)
        n_blk = B * N_SK
        v_sb = [
            persist.tile([128, VSTRIDE * n_blk], V_DT, tag=f"v{h}", name=f"v_sb{h}")
            for h in range(HPC)
        ]
        for h in range(HPC):
            nc.vector.memset(v_sb[h], 1.0)
        # normalized y.T: rows = 2 heads x 64 dims (bf16)
        ynorm = persist.tile([128, SQ], BF16, tag="ynorm")

        # ---------- emission units ----------

        def prep_chunk_units(b, j):
            """DMA + transpose + qkv + v-naturalize for 512-row chunk j of
            batch b. Yields after each schedulable sub-unit."""
            row0 = S * b + 512 * j
            xrows = []
            for p in range(4):
                xr = xrow_pool.tile([128, C], BF16, name=f"xr_{b}_{j}_{p}", tag="xr")
                nc.sync.dma_start(
                    out=xr, in_=x[row0 + 128 * p:row0 + 128 * (p + 1), :]
                )
                xrows.append(xr)
            yield
            # transpose x chunk: xt[k] = [128 C-dims, 512 sq] bf16
            xts = []
            for k in range(C // 128):
                tp = ps_u.tile([128, 512], F32, name=f"tp_{b}_{j}_{k}", tag="u512")
                for p in range(4):
                    nc.tensor.matmul(
                        tp[:, 128 * p:128 * (p + 1)],
                        xrows[p][:, 128 * k:128 * (k + 1)],
                        identity,
                        start=True, stop=True,
                    )
                xt = xt_pool.tile([128, 512], BF16, name=f"xt_{b}_{j}_{k}", tag="xt")
                nc.vector.tensor_copy(xt, tp)
                xts.append(xt)
                if k % 2 == 1:
                    yield
            # qkv matmuls: qkvT[m][:, chunk] = w_m.T @ x.T + b
            for m in range(3):
                qp = ps_u.tile([128, 512], F32, name=f"qp_{b}_{j}_{m}", tag="u512")
                for k in range(C // 128):
                    nc.tensor.matmul(
                        qp,
                        w_sb[k][:, 128 * m:128 * (m + 1)],
                        xts[k],
                        start=(k == 0),
                        stop=(k == C // 128 - 1),
                    )
                col0 = row0  # global sq column of this chunk
                nc.scalar.activation(
                    qkvT[m][:, col0:col0 + 512], qp,
                    mybir.ActivationFunctionType.Identity,
                    bias=battn_sb[:, m:m + 1],
                )
                yield
            # v natural layout: per 128-sk block, v_nat = (v.T block).T via PE
            tpv = ps_u.tile([128, 512], F32, name=f"tpv_{b}_{j}", tag="u512")
            for p in range(4):
                blk = N_SK * b + 4 * j + p
                nc.tensor.matmul(
                    tpv[:, 128 * p:128 * (p + 1)],
                    qkvT[2][:, 128 * blk:128 * (blk + 1)],
                    identity,
                    start=True, stop=True,
                )
            # scatter into v_sb (stride VSTRIDE), cast fp8, per head
            for h in range(HPC):
                src = (tpv.rearrange("a (n c) -> a n c", c=128)
                       [:, :, 64 * h:64 * h + 64])
                blk0 = N_SK * b + 4 * j
                dst = (
                    v_sb[h][:, VSTRIDE * blk0:VSTRIDE * (blk0 + 4)]
                    .rearrange("a (n c) -> a n c", c=VSTRIDE)[:, :, 0:64]
                )
                nc.vector.tensor_copy(dst, src)
            yield

        def attn_pair_units(b, j4):
            """Attention for sq-chunk (b, j4): loop over sk tile-pairs."""
            ni = 4 * j4 + 4
            col0 = S * b + 512 * j4
            yps = [
                ps_y.tile([128, 512], F32, name=f"y_{b}_{j4}_{h}", tag="yps")
                for h in range(HPC)
            ]
            n_pair = ni // 2
            for pr in range(n_pair):
                i0 = 2 * pr
                pts = []
                for h in range(HPC):
                    sp = ps_sp.tile([128, 1024], F32,
                                    name=f"s_{b}_{j4}_{pr}_{h}", tag="sp")
                    for jj in range(2):
                        i = i0 + jj
                        nc.tensor.matmul(
                            sp[:, 512 * jj:512 * (jj + 1)],
                            qkvT[1][64 * h:64 * (h + 1),
                                    S * b + 128 * i:S * b + 128 * (i + 1)],
                            qkvT[0][64 * h:64 * (h + 1), col0:col0 + 512],
                            start=True, stop=True,
                        )
                    pt = pt_pool.tile([128, 1024], PT_DT,
                                      name=f"pt_{b}_{j4}_{pr}_{h}", tag="ptt")
                    # bias -2 keeps exp within fp8 e4m3 range (max 240, infs
                    # past ~5.5 sigma scores); softmax-invariant since the
                    # ones-column denominators scale by the same e^-2.
                    nc.scalar.activation(
                        pt, sp, mybir.ActivationFunctionType.Exp,
                        scale=0.125, bias=expbias[:, 0:1],
                    )
                    if 128 * (i0 + 1) + 127 > 512 * j4:  # pair straddles diag
                        ptv = pt.rearrange("a (n c) -> a n c", c=512)
                        nc.gpsimd.affine_select(
                            out=ptv, in_=ptv,
                            pattern=[[-128, 2], [1, 512]],
                            channel_multiplier=-1,
                            base=512 * j4 - 128 * i0,
                            compare_op=mybir.AluOpType.is_ge,
                            fill=0.0,
                        )
                    if DEBUG and (b, j4, pr, h) == (0, 0, 0, 0):
                        nc.sync.dma_start(out=dbg_pt, in_=pt)
                    pts.append(pt)
                for h in range(HPC):
                    blk0 = N_SK * b + i0
                    if USE_DR:
                        vv = (
                            v_sb[h][:, VSTRIDE * blk0:VSTRIDE * (blk0 + 2)]
                            .rearrange("a (n c) -> a n c", c=VSTRIDE)[:, :, 0:65]
                        )
                        nc.tensor.matmul(
                            yps[h][0:65, :],
                            vv,
                            pts[h].rearrange("a (n c) -> a n c", c=512),
                            start=(pr == 0),
                            stop=(pr == n_pair - 1),
                            perf_mode=mybir.MatmulPerfMode.DoubleRow,
                        )
                    else:
                        for jj in range(2):
                            blk = blk0 + jj
                            vv = v_sb[h][:, VSTRIDE * blk:VSTRIDE * blk + 65]
                            nc.tensor.matmul(
                                yps[h][0:65, :],
                                vv,
                                pts[h][:, 512 * jj:512 * (jj + 1)],
                                start=(pr == 0 and jj == 0),
                                stop=(pr == n_pair - 1 and jj == 1),
                            )
                yield
            # softmax normalization per head
            for h in range(HPC):
                recip = small_pool.tile([1, 512], F32,
                                        name=f"rc_{b}_{j4}_{h}", tag="rc")
                nc.vector.reciprocal(recip, yps[h][64:65, :])
                if DEBUG:
                    idx = (b * N_CH + j4) * HPC + h
                    nc.sync.dma_start(out=dbg_rc[idx:idx + 1, :], in_=recip)
                bc = small_pool.tile([64, 512], F32,
                                     name=f"bc_{b}_{j4}_{h}", tag="bc")
                nc.gpsimd.partition_broadcast(bc, recip)
                nc.vector.tensor_mul(
                    ynorm[64 * h:64 * (h + 1), col0:col0 + 512],
                    yps[h][0:64, :],
                    bc,
                )
            yield

        def proj_units(b, j4):
            """Projection + output DMA for sq-chunk (b, j4)."""
            for t in range(4):
                r0 = S * b + 512 * j4 + 128 * t
                ob = ob_pool.tile([128, C], BF16, name=f"ob_{b}_{j4}_{t}", tag="ob")
                for n in range(2):
                    pp = ps_u.tile([128, 512], F32,
                                   name=f"pp_{b}_{j4}_{t}_{n}", tag="u512")
                    nc.tensor.matmul(
                        pp,
                        ynorm[:, r0:r0 + 128],
                        wproj_sb[:, 512 * n:512 * (n + 1)],
                        start=True, stop=True,
                    )
                    nc.vector.tensor_copy(ob[:, 512 * n:512 * (n + 1)], pp)
                nc.sync.dma_start(out=out[r0:r0 + 128, :], in_=ob)
                if t % 2 == 1:
                    yield

        # ---------- scheduler ----------
        prep_q = deque()
        for b in range(B):
            for j in range(N_CH):
                prep_q.append((b, j, prep_chunk_units(b, j)))
        prep_done = set()
        work_q = deque()  # proj generators

        def pump_prep():
            """Advance the frontmost prep generator by one unit."""
            if not prep_q:
                return False
            b, j, g = prep_q[0]
            try:
                next(g)
            except StopIteration:
                prep_done.add((b, j))
                prep_q.popleft()
            return True

        def pump_work():
            if not work_q:
                return False
            g = work_q[0]
            try:
                next(g)
            except StopIteration:
                work_q.popleft()
            return True

        def pump_background(n):
            for _ in range(n):
                if not pump_prep():
                    if not pump_work():
                        return

        for b in range(B):
            for j4 in range(N_CH):
                while (b, j4) not in prep_done:
                    pump_prep()
                ag = attn_pair_units(b, j4)
                for _ in ag:
                    pump_background(2)
                work_q.append(proj_units(b, j4))
                pump_background(1)
        while work_q or prep_q:
            if not pump_work():
                pump_prep()
        if DEBUG:
            nc.sync.dma_start(out=dbg_v, in_=v_sb[0])

    nc.compile()
    return nc


def shard_inputs(x, w_attn, b_attn, w_proj):
    """Build the 8 per-core input maps (bf16 weights/activations)."""
    import ml_dtypes

    bf16 = ml_dtypes.bfloat16
    xf = np.ascontiguousarray(
        np.asarray(x, dtype=np.float32).reshape(-1, C)
    ).astype(bf16)
    w_attn = np.asarray(w_attn, dtype=np.float32)
    b_attn = np.asarray(b_attn, dtype=np.float32)
    w_proj = np.asarray(w_proj, dtype=np.float32)
    in_maps = []
    for c in range(N_CORES):
        heads = [HPC * c + h for h in range(HPC)]
        cols = []
        for part in range(3):  # q, k, v
            for h in heads:
                cols.append(np.arange(part * C + D * h, part * C + D * (h + 1)))
        cols = np.concatenate(cols)
        w_qkv_c = np.ascontiguousarray(w_attn[:, cols]).astype(bf16)
        b_qkv_c = np.ascontiguousarray(b_attn[cols].reshape(-1, 1))
        w_proj_c = np.ascontiguousarray(
            w_proj[D * heads[0]:D * (heads[-1] + 1), :]
        ).astype(bf16)
        in_maps.append(
            {"x": xf, "w_qkv": w_qkv_c, "b_qkv": b_qkv_c, "w_proj": w_proj_c}
        )
    return in_maps


def kernel(x, w_attn, b_attn, w_proj, b_proj):
    global LAST_EXEC_NS, LAST_RES
    x = np.asarray(x, dtype=np.float32)
    Bv, Sv, Cv = x.shape
    assert (Bv, Sv, Cv) == (B, S, C), (Bv, Sv, Cv)
    nc = build_nc()
    in_maps = shard_inputs(x, w_attn, b_attn, w_proj)
    extra = {}
    if os.environ.get("ATTN_TMPDIR"):
        extra["tmpdir"] = os.environ["ATTN_TMPDIR"]
    res = run_bass_kernel_spmd(nc, in_maps, list(range(N_CORES)), **extra)
    LAST_EXEC_NS = res.exec_time_ns
    LAST_RES = res
    acc = np.zeros((B * S, C), dtype=np.float32)
    for r in res.results:
        acc += np.asarray(r["out"]).astype(np.float32)
    acc += np.asarray(b_proj, dtype=np.float32)[None, :]
    return acc.reshape(B, S, C)


# revision 35
# speedup vs baseline: 1.1916x; 1.1087x over previous
"""Causal self-attention (B=2, S=2048, C=1024, H=16) on 8 TRN2 NeuronCores.

Sharding: tensor-parallel over heads - 2 heads per core. Each core computes
  qkv.T = w_c.T @ x.T        (its 384 qkv columns, bf16, x transposed on PE
                              via regular matmuls against identity)
  scores.T = k @ q.T         (per head, [sk, sq] bf16, causal pair-blocked)
  P.T = exp(scores.T / 8)    (fp8 e4m3; no max-subtraction; scores ~ N(0,1))
  y_aug.T = [v | 1].T @ P.T  (fp8 DoubleRow, row 64 = softmax denominators)
  y_norm.T = y.T * (1/sums)  (reciprocal on [1,512], broadcast via PE matmul)
  out_partial = y_norm @ w_proj_c  ([4096, 1024] bf16 partial over head dims)
Host sums the 8 partials (upcast) and adds biases (b_attn folded on-device).

Emission interleaves qkv-prep / attention / projection so the scalar-engine
exp stream overlaps PE work and DMA stays busy throughout.
"""

import os
from collections import deque
from contextlib import ExitStack

import numpy as np

import concourse.bass as bass
import concourse.tile as tile
from concourse import bacc, mybir
from concourse.bass_utils import run_bass_kernel_spmd
from concourse.masks import make_identity

F32 = mybir.dt.float32
F32R = mybir.dt.float32r
BF16 = mybir.dt.bfloat16
FP8 = mybir.dt.float8e4
FP8E5 = mybir.dt.float8e5

N_HEAD = 16
N_EMBD = 1024
B = 2
S = 2048
C = N_EMBD
D = C // N_HEAD  # 64
N_CORES = 8
HPC = N_HEAD // N_CORES  # 2 heads per core

SQ = B * S            # 4096 flattened rows
N_CH = S // 512       # 512-row chunks per batch
N_SK = S // 128       # 128-row sk tiles per batch
W_COLS = 3 * HPC * D  # 384
VSTRIDE = 80          # fp8 v block stride: 64 v + 1 ones + 15 pad (16B align)

LAST_EXEC_NS = None
LAST_RES = None

USE_FP8 = os.environ.get("ATTN_FP8", "0") == "1"
USE_DR = USE_FP8 and os.environ.get("ATTN_DR", "1") == "1"
# P in e5m2: huge dynamic range (max 57344, subnormals to 2^-16) so plain
# exp(s/8) fits with no bias shift; V in e4m3 for its 3-bit mantissa.
PT_DT = FP8E5 if USE_FP8 else BF16
V_DT = FP8 if USE_FP8 else BF16


def build_nc():
    nc = bacc.Bacc("TRN2", target_bir_lowering=False, debug=False)

    x = nc.dram_tensor("x", [SQ, C], BF16, kind="ExternalInput").ap()
    w_qkv = nc.dram_tensor("w_qkv", [C, W_COLS], BF16, kind="ExternalInput").ap()
    b_qkv = nc.dram_tensor("b_qkv", [W_COLS, 1], F32, kind="ExternalInput").ap()
    w_proj = nc.dram_tensor("w_proj", [HPC * D, C], BF16, kind="ExternalInput").ap()
    out = nc.dram_tensor("out", [SQ, C], BF16, kind="ExternalOutput").ap()
    DEBUG = os.environ.get("ATTN_DEBUG", "0") == "1"
    if DEBUG:
        dbg_v = nc.dram_tensor(
            "dbg_v", [128, VSTRIDE * B * N_SK], PT_DT, kind="ExternalOutput"
        ).ap()
        dbg_pt = nc.dram_tensor(
            "dbg_pt", [128, 1024], PT_DT, kind="ExternalOutput"
        ).ap()
        dbg_rc = nc.dram_tensor(
            "dbg_rc", [16, 512], F32, kind="ExternalOutput"
        ).ap()

    with tile.TileContext(nc) as tc, ExitStack() as ctx:
        persist = ctx.enter_context(tc.tile_pool(name="persist", bufs=1))
        xrow_pool = ctx.enter_context(tc.tile_pool(name="xrow", bufs=8))
        xt_pool = ctx.enter_context(tc.tile_pool(name="xt", bufs=16))
        pt_pool = ctx.enter_context(tc.tile_pool(name="pt", bufs=4))
        small_pool = ctx.enter_context(tc.tile_pool(name="small", bufs=4))
        ob_pool = ctx.enter_context(tc.tile_pool(name="ob", bufs=4))
        ps_sp = ctx.enter_context(tc.tile_pool(name="ps_sp", bufs=2, space="PSUM"))
        ps_y = ctx.enter_context(tc.tile_pool(name="ps_y", bufs=2, space="PSUM"))
        ps_u = ctx.enter_context(tc.tile_pool(name="ps_u", bufs=2, space="PSUM"))

        # ---- persistent sbuf ----
        identity = persist.tile([128, 128], BF16, tag="identity")
        make_identity(nc, identity)

        # w DMAs are issued by load_weights() AFTER the first x chunk's DMAs
        # so the PE's first transposes aren't stuck behind them on the queue.
        w_sb = []
        battn_sb = persist.tile([128, 3], F32, tag="battn")
        wproj_sb = persist.tile([128, C], BF16, tag="wproj")
        expbias = persist.tile([128, 1], F32, tag="expbias")

        def load_weights():
            for k in range(C // 128):
                wt = persist.tile([128, W_COLS], BF16, tag=f"w{k}",
                                  name=f"w_sb{k}")
                nc.sync.dma_start(out=wt, in_=w_qkv[128 * k:128 * (k + 1), :])
                w_sb.append(wt)
            for m in range(3):
                nc.sync.dma_start(
                    out=battn_sb[:, m:m + 1], in_=b_qkv[128 * m:128 * (m + 1), :]
                )
            nc.sync.dma_start(out=wproj_sb, in_=w_proj)
            nc.vector.memset(expbias, 0.0)

        # qkv.T: [0]=q.T, [1]=k.T (bf16), [2]=v.T (bf16, transposed back later)
        qkvT = [
            persist.tile([128, SQ], BF16, tag=f"qkvT{m}", name=f"qkvT{m}")
            for m in range(3)
        ]
        # v in natural layout fp8, augmented ones column at offset 64 of each
        # VSTRIDE block (whole tile memset to 1; v blocks overwrite cols 0-63)
        n_blk = B * N_SK
        v_sb = [
            persist.tile([128, VSTRIDE * n_blk], V_DT, tag=f"v{h}", name=f"v_sb{h}")
            for h in range(HPC)
        ]
        # normalized y.T: rows = 2 heads x 64 dims (bf16)
        ynorm = persist.tile([128, SQ], BF16, tag="ynorm")

        # ---------- emission units ----------

        def prep_chunk_units(b, j):
            """DMA + transpose + qkv + v-naturalize for 512-row chunk j of
            batch b. Yields after each schedulable sub-unit."""
            row0 = S * b + 512 * j
            xrows = []
            for p in range(4):
                xr = xrow_pool.tile([128, C], BF16, name=f"xr_{b}_{j}_{p}", tag="xr")
                nc.sync.dma_start(
                    out=xr, in_=x[row0 + 128 * p:row0 + 128 * (p + 1), :]
                )
                xrows.append(xr)
            yield
            # transpose x chunk: xt[k] = [128 C-dims, 512 sq] bf16
            xts = []
            for k in range(C // 128):
                tp = ps_u.tile([128, 512], F32, name=f"tp_{b}_{j}_{k}", tag="u512")
                for p in range(4):
                    nc.tensor.matmul(
                        tp[:, 128 * p:128 * (p + 1)],
                        xrows[p][:, 128 * k:128 * (k + 1)],
                        identity,
                        start=True, stop=True,
                    )
                xt = xt_pool.tile([128, 512], BF16, name=f"xt_{b}_{j}_{k}", tag="xt")
                nc.vector.tensor_copy(xt, tp)
                xts.append(xt)
                if k % 2 == 1:
                    yield
            # qkv matmuls: qkvT[m][:, chunk] = w_m.T @ x.T + b
            for m in range(3):
                qp = ps_u.tile([128, 512], F32, name=f"qp_{b}_{j}_{m}", tag="u512")
                for k in range(C // 128):
                    nc.tensor.matmul(
                        qp,
                        w_sb[k][:, 128 * m:128 * (m + 1)],
                        xts[k],
                        start=(k == 0),
                        stop=(k == C // 128 - 1),
                    )
                col0 = row0  # global sq column of this chunk
                nc.scalar.activation(
                    qkvT[m][:, col0:col0 + 512], qp,
                    mybir.ActivationFunctionType.Identity,
                    bias=battn_sb[:, m:m + 1],
                )
                yield
            # v natural layout: per 128-sk block, v_nat = (v.T block).T via PE
            tpv = ps_u.tile([128, 512], F32, name=f"tpv_{b}_{j}", tag="u512")
            for p in range(4):
                blk = N_SK * b + 4 * j + p
                nc.tensor.matmul(
                    tpv[:, 128 * p:128 * (p + 1)],
                    qkvT[2][:, 128 * blk:128 * (blk + 1)],
                    identity,
                    start=True, stop=True,
                )
            # scatter into v_sb (stride VSTRIDE), cast fp8, per head
            for h in range(HPC):
                src = (tpv.rearrange("a (n c) -> a n c", c=128)
                       [:, :, 64 * h:64 * h + 64])
                blk0 = N_SK * b + 4 * j
                dst = (
                    v_sb[h][:, VSTRIDE * blk0:VSTRIDE * (blk0 + 4)]
                    .rearrange("a (n c) -> a n c", c=VSTRIDE)[:, :, 0:64]
                )
                nc.vector.tensor_copy(dst, src)
            yield

        def attn_pair_units(b, j4):
            """Attention for sq-chunk (b, j4): loop over sk tile-pairs.

            y matmuls are emitted one pair behind the scores/exp so the
            in-order PE queue never blocks on the Act-engine exp."""
            ni = 4 * j4 + 4
            col0 = S * b + 512 * j4
            yps = [
                ps_y.tile([128, 512], F32, name=f"y_{b}_{j4}_{h}", tag="yps")
                for h in range(HPC)
            ]
            n_pair = ni // 2

            def emit_y(pt2, pr):
                i0 = 2 * pr
                for h in range(HPC):
                    blk0 = N_SK * b + i0
                    if USE_DR:
                        vv = (
                            v_sb[h][:, VSTRIDE * blk0:VSTRIDE * (blk0 + 2)]
                            .rearrange("a (n c) -> a n c", c=VSTRIDE)[:, :, 0:65]
                        )
                        nc.tensor.matmul(
                            yps[h][0:65, :],
                            vv,
                            pt2[:, 1024 * h:1024 * (h + 1)]
                            .rearrange("a (n c) -> a n c", c=512),
                            start=(pr == 0),
                            stop=(pr == n_pair - 1),
                            perf_mode=mybir.MatmulPerfMode.DoubleRow,
                        )
                    else:
                        for jj in range(2):
                            blk = blk0 + jj
                            vv = v_sb[h][:, VSTRIDE * blk:VSTRIDE * blk + 65]
                            nc.tensor.matmul(
                                yps[h][0:65, :],
                                vv,
                                pt2[:, 1024 * h + 512 * jj:
                                    1024 * h + 512 * (jj + 1)],
                                start=(pr == 0 and jj == 0),
                                stop=(pr == n_pair - 1 and jj == 1),
                            )

            prev = None
            for pr in range(n_pair):
                i0 = 2 * pr
                # both heads' P tiles side by side: [h0 i0|i1, h1 i0|i1]
                pt2 = pt_pool.tile([128, 2048], PT_DT,
                                   name=f"pt_{b}_{j4}_{pr}", tag="ptt")
                for h in range(HPC):
                    sp = ps_sp.tile([128, 1024], F32,
                                    name=f"s_{b}_{j4}_{pr}_{h}", tag="sp")
                    for jj in range(2):
                        i = i0 + jj
                        nc.tensor.matmul(
                            sp[:, 512 * jj:512 * (jj + 1)],
                            qkvT[1][64 * h:64 * (h + 1),
                                    S * b + 128 * i:S * b + 128 * (i + 1)],
                            qkvT[0][64 * h:64 * (h + 1), col0:col0 + 512],
                            start=True, stop=True,
                        )
                    nc.scalar.activation(
                        pt2[:, 1024 * h:1024 * (h + 1)], sp,
                        mybir.ActivationFunctionType.Exp,
                        scale=0.125, bias=expbias[:, 0:1],
                    )
                if 128 * (i0 + 1) + 127 > 512 * j4:  # pair straddles diag
                    ptv = pt2.rearrange("a (h n c) -> a h n c", h=2, c=512)
                    nc.gpsimd.affine_select(
                        out=ptv, in_=ptv,
                        pattern=[[0, 2], [-128, 2], [1, 512]],
                        channel_multiplier=-1,
                        base=512 * j4 - 128 * i0,
                        compare_op=mybir.AluOpType.is_ge,
                        fill=0.0,
                    )
                if DEBUG and (b, j4, pr) == (0, 0, 0):
                    nc.sync.dma_start(out=dbg_pt, in_=pt2[:, 0:1024])
                if prev is not None:
                    emit_y(*prev)
                prev = (pt2, pr)
                yield
            emit_y(*prev)
            # softmax normalization: fast reciprocal of both heads' sums,
            # one partition_broadcast for both, then per-head multiply
            sums_sb = small_pool.tile([1, 1024], F32,
                                      name=f"sm_{b}_{j4}", tag="sm")
            for h in range(HPC):
                nc.vector.tensor_copy(
                    sums_sb[:, 512 * h:512 * (h + 1)], yps[h][64:65, :]
                )
            recip = small_pool.tile([1, 1024], F32,
                                    name=f"rc_{b}_{j4}", tag="rc")
            nc.vector.reciprocal_approx_fast(out=recip, in_=sums_sb)
            if DEBUG:
                idx = (b * N_CH + j4)
                nc.sync.dma_start(out=dbg_rc[2 * idx:2 * idx + 1, :],
                                  in_=recip[:, 0:512])
            bc = small_pool.tile([64, 1024], F32,
                                 name=f"bc_{b}_{j4}", tag="bc")
            nc.gpsimd.partition_broadcast(bc, recip)
            for h in range(HPC):
                nc.vector.tensor_mul(
                    ynorm[64 * h:64 * (h + 1), col0:col0 + 512],
                    yps[h][0:64, :],
                    bc[:, 512 * h:512 * (h + 1)],
                )
            yield

        def proj_units(b, j4):
            """Projection + output DMA for sq-chunk (b, j4)."""
            for t in range(4):
                r0 = S * b + 512 * j4 + 128 * t
                ob = ob_pool.tile([128, C], BF16, name=f"ob_{b}_{j4}_{t}", tag="ob")
                for n in range(2):
                    pp = ps_u.tile([128, 512], F32,
                                   name=f"pp_{b}_{j4}_{t}_{n}", tag="u512")
                    nc.tensor.matmul(
                        pp,
                        ynorm[:, r0:r0 + 128],
                        wproj_sb[:, 512 * n:512 * (n + 1)],
                        start=True, stop=True,
                    )
                    nc.vector.tensor_copy(ob[:, 512 * n:512 * (n + 1)], pp)
                nc.sync.dma_start(out=out[r0:r0 + 128, :], in_=ob)
                if t % 2 == 1:
                    yield

        # ---------- scheduler ----------
        prep_q = deque()
        for b in range(B):
            for j in range(N_CH):
                prep_q.append((b, j, prep_chunk_units(b, j)))
        prep_done = set()
        work_q = deque()  # proj generators

        def pump_prep():
            """Advance the frontmost prep generator by one unit."""
            if not prep_q:
                return False
            b, j, g = prep_q[0]
            try:
                next(g)
            except StopIteration:
                prep_done.add((b, j))
                prep_q.popleft()
            return True

        def pump_work():
            if not work_q:
                return False
            g = work_q[0]
            try:
                next(g)
            except StopIteration:
                work_q.popleft()
            return True

        def pump_background(n):
            # prefer prep; hold the most recent proj generator in reserve so
            # the final (longest) attention chunk has PE work to hide its exp
            for _ in range(n):
                if pump_prep():
                    continue
                if len(work_q) > 1:
                    pump_work()
                else:
                    return

        # first x chunk's DMAs go out before the weight DMAs
        first_prep = prep_chunk_units(0, 0)
        next(first_prep)
        load_weights()
        for h in range(HPC):
            nc.vector.memset(v_sb[h], 1.0)
        prep_q[0] = (0, 0, first_prep)

        for b in range(B):
            for j4 in range(N_CH):
                while (b, j4) not in prep_done:
                    pump_prep()
                ag = attn_pair_units(b, j4)
                for _ in ag:
                    pump_background(2)
                work_q.append(proj_units(b, j4))
                pump_background(1)
        while work_q or prep_q:
            if not pump_work():
                pump_prep()
        if DEBUG:
            nc.sync.dma_start(out=dbg_v, in_=v_sb[0])

    nc.compile()
    return nc


def shard_inputs(x, w_attn, b_attn, w_proj):
    """Build the 8 per-core input maps (bf16 weights/activations)."""
    import ml_dtypes

    bf16 = ml_dtypes.bfloat16
    xf = np.ascontiguousarray(
        np.asarray(x, dtype=np.float32).reshape(-1, C)
    ).astype(bf16)
    w_attn = np.asarray(w_attn, dtype=np.float32)
    b_attn = np.asarray(b_attn, dtype=np.float32)
    w_proj = np.asarray(w_proj, dtype=np.float32)
    in_maps = []
    for c in range(N_CORES):
        heads = [HPC * c + h for h in range(HPC)]
        cols = []
        for part in range(3):  # q, k, v
            for h in heads:
                cols.append(np.arange(part * C + D * h, part * C + D * (h + 1)))
        cols = np.concatenate(cols)
        w_qkv_c = np.ascontiguousarray(w_attn[:, cols]).astype(bf16)
        b_qkv_c = np.ascontiguousarray(b_attn[cols].reshape(-1, 1))
        w_proj_c = np.ascontiguousarray(
            w_proj[D * heads[0]:D * (heads[-1] + 1), :]
        ).astype(bf16)
        in_maps.append(
            {"x": xf, "w_qkv": w_qkv_c, "b_qkv": b_qkv_c, "w_proj": w_proj_c}
        )
    return in_maps


def kernel(x, w_attn, b_attn, w_proj, b_proj):
    global LAST_EXEC_NS, LAST_RES
    x = np.asarray(x, dtype=np.float32)
    Bv, Sv, Cv = x.shape
    assert (Bv, Sv, Cv) == (B, S, C), (Bv, Sv, Cv)
    nc = build_nc()
    in_maps = shard_inputs(x, w_attn, b_attn, w_proj)
    extra = {}
    if os.environ.get("ATTN_TMPDIR"):
        extra["tmpdir"] = os.environ["ATTN_TMPDIR"]
    res = run_bass_kernel_spmd(nc, in_maps, list(range(N_CORES)), **extra)
    LAST_EXEC_NS = res.exec_time_ns
    LAST_RES = res
    acc = np.zeros((B * S, C), dtype=np.float32)
    for r in res.results:
        acc += np.asarray(r["out"]).astype(np.float32)
    acc += np.asarray(b_proj, dtype=np.float32)[None, :]
    return acc.reshape(B, S, C)


# revision 42
# speedup vs baseline: 1.5499x; 1.3007x over previous
"""Causal self-attention (B=2, S=2048, C=1024, H=16) on 8 TRN2 NeuronCores.

Sharding: tensor-parallel over heads - 2 heads per core. Each core computes
  qkv.T = w_c.T @ x.T        (its 384 qkv columns, bf16, x transposed on PE
                              via regular matmuls against identity)
  scores.T = k @ q.T         (per head, [sk, sq] bf16, causal pair-blocked)
  P.T = exp(scores.T / 8)    (fp8 e4m3; no max-subtraction; scores ~ N(0,1))
  y_aug.T = [v | 1].T @ P.T  (fp8 DoubleRow, row 64 = softmax denominators)
  y_norm.T = y.T * (1/sums)  (reciprocal on [1,512], broadcast via PE matmul)
  out_partial = y_norm @ w_proj_c  ([4096, 1024] bf16 partial over head dims)
Host sums the 8 partials (upcast) and adds biases (b_attn folded on-device).

Emission interleaves qkv-prep / attention / projection so the scalar-engine
exp stream overlaps PE work and DMA stays busy throughout.
"""

import os
from collections import deque
from contextlib import ExitStack

import numpy as np

import concourse.bass as bass
import concourse.tile as tile
from concourse import bacc, mybir
from concourse.bass_utils import run_bass_kernel_spmd
from concourse.masks import make_identity

F32 = mybir.dt.float32
F32R = mybir.dt.float32r
BF16 = mybir.dt.bfloat16
FP8 = mybir.dt.float8e4
FP8E5 = mybir.dt.float8e5

N_HEAD = 16
N_EMBD = 1024
B = 2
S = 2048
C = N_EMBD
D = C // N_HEAD  # 64
N_CORES = 8
HPC = N_HEAD // N_CORES  # 2 heads per core

SQ = B * S            # 4096 flattened rows
N_CH = S // 512       # 512-row chunks per batch
N_SK = S // 128       # 128-row sk tiles per batch
W_COLS = 3 * HPC * D  # 384
VSTRIDE = 80          # fp8 v block stride: 64 v + 1 ones + 15 pad (16B align)

LAST_EXEC_NS = None
LAST_RES = None

USE_FP8 = os.environ.get("ATTN_FP8", "0") == "1"
USE_DR = USE_FP8 and os.environ.get("ATTN_DR", "1") == "1"
# P in e5m2: huge dynamic range (max 57344, subnormals to 2^-16) so plain
# exp(s/8) fits with no bias shift; V in e4m3 for its 3-bit mantissa.
PT_DT = FP8E5 if USE_FP8 else BF16
V_DT = FP8 if USE_FP8 else BF16


def build_nc():
    nc = bacc.Bacc("TRN2", target_bir_lowering=False, debug=False)

    x = nc.dram_tensor("x", [SQ, C], BF16, kind="ExternalInput").ap()
    w_qkv = nc.dram_tensor("w_qkv", [C, W_COLS], BF16, kind="ExternalInput").ap()
    b_qkv = nc.dram_tensor("b_qkv", [W_COLS, 1], F32, kind="ExternalInput").ap()
    w_proj = nc.dram_tensor("w_proj", [HPC * D, C], BF16, kind="ExternalInput").ap()
    out = nc.dram_tensor("out", [SQ, C], BF16, kind="ExternalOutput").ap()
    DEBUG = os.environ.get("ATTN_DEBUG", "0") == "1"
    if DEBUG:
        dbg_v = nc.dram_tensor(
            "dbg_v", [128, VSTRIDE * B * N_SK], PT_DT, kind="ExternalOutput"
        ).ap()
        dbg_pt = nc.dram_tensor(
            "dbg_pt", [128, 1024], PT_DT, kind="ExternalOutput"
        ).ap()
        dbg_rc = nc.dram_tensor(
            "dbg_rc", [16, 512], F32, kind="ExternalOutput"
        ).ap()

    with tile.TileContext(nc) as tc, ExitStack() as ctx:
        persist = ctx.enter_context(tc.tile_pool(name="persist", bufs=1))
        xt_pool = ctx.enter_context(tc.tile_pool(name="xt", bufs=3))
        pt_pool = ctx.enter_context(tc.tile_pool(name="pt", bufs=4))
        small_pool = ctx.enter_context(tc.tile_pool(name="small", bufs=4))
        ob_pool = ctx.enter_context(tc.tile_pool(name="ob", bufs=4))
        ps_sp = ctx.enter_context(tc.tile_pool(name="ps_sp", bufs=2, space="PSUM"))
        ps_y = ctx.enter_context(tc.tile_pool(name="ps_y", bufs=2, space="PSUM"))
        ps_u = ctx.enter_context(tc.tile_pool(name="ps_u", bufs=2, space="PSUM"))

        # ---- persistent sbuf ----
        identity = persist.tile([128, 128], BF16, tag="identity")
        make_identity(nc, identity)

        # w DMAs are issued by load_weights() AFTER the first x chunk's DMAs
        # so the PE's first transposes aren't stuck behind them on the queue.
        w_sb = []
        battn_sb = persist.tile([128, 3], F32, tag="battn")
        wproj_sb = persist.tile([128, C], BF16, tag="wproj")
        expbias = persist.tile([128, 1], F32, tag="expbias")

        def load_weights():
            for k in range(C // 128):
                wt = persist.tile([128, W_COLS], BF16, tag=f"w{k}",
                                  name=f"w_sb{k}")
                nc.sync.dma_start(out=wt, in_=w_qkv[128 * k:128 * (k + 1), :])
                w_sb.append(wt)
            for m in range(3):
                nc.sync.dma_start(
                    out=battn_sb[:, m:m + 1], in_=b_qkv[128 * m:128 * (m + 1), :]
                )
            nc.sync.dma_start(out=wproj_sb, in_=w_proj)
            nc.vector.memset(expbias, 0.0)

        # qkv.T: [0]=q.T, [1]=k.T (bf16), [2]=v.T (bf16, transposed back later)
        qkvT = [
            persist.tile([128, SQ], BF16, tag=f"qkvT{m}", name=f"qkvT{m}")
            for m in range(3)
        ]
        # v in natural layout fp8, augmented ones column at offset 64 of each
        # VSTRIDE block (whole tile memset to 1; v blocks overwrite cols 0-63)
        n_blk = B * N_SK
        v_sb = [
            persist.tile([128, VSTRIDE * n_blk], V_DT, tag=f"v{h}", name=f"v_sb{h}")
            for h in range(HPC)
        ]
        # normalized y.T: rows = 2 heads x 64 dims (bf16)
        ynorm = persist.tile([128, SQ], BF16, tag="ynorm")

        # ---------- emission units ----------

        def prep_chunk_units(b, j):
            """DMA-transpose + qkv + v-naturalize for 512-row chunk j of
            batch b. Yields after each schedulable sub-unit."""
            row0 = S * b + 512 * j
            # x chunk transposed on the fly by the DMA XBAR:
            # xt[p, k, c] = x[row0 + c, 128k + p]
            xt = xt_pool.tile([128, C // 128, 512], BF16,
                              name=f"xt_{b}_{j}", tag="xt")
            nc.sync.dma_start_transpose(out=xt, in_=x[row0:row0 + 512, :])
            yield
            # qkv matmuls: qkvT[m][:, chunk] = w_m.T @ x.T + b
            for m in range(3):
                qp = ps_u.tile([128, 512], F32, name=f"qp_{b}_{j}_{m}", tag="u512")
                for k in range(C // 128):
                    nc.tensor.matmul(
                        qp,
                        w_sb[k][:, 128 * m:128 * (m + 1)],
                        xt[:, k, :],
                        start=(k == 0),
                        stop=(k == C // 128 - 1),
                    )
                col0 = row0  # global sq column of this chunk
                nc.vector.tensor_scalar_add(
                    qkvT[m][:, col0:col0 + 512], qp, battn_sb[:, m:m + 1]
                )
                yield
            # v natural layout: per 128-sk block, v_nat = (v.T block).T via PE
            tpv = ps_u.tile([128, 512], F32, name=f"tpv_{b}_{j}", tag="u512")
            for p in range(4):
                blk = N_SK * b + 4 * j + p
                nc.tensor.matmul(
                    tpv[:, 128 * p:128 * (p + 1)],
                    qkvT[2][:, 128 * blk:128 * (blk + 1)],
                    identity,
                    start=True, stop=True,
                )
            # scatter into v_sb (stride VSTRIDE), cast fp8, per head
            for h in range(HPC):
                src = (tpv.rearrange("a (n c) -> a n c", c=128)
                       [:, :, 64 * h:64 * h + 64])
                blk0 = N_SK * b + 4 * j
                dst = (
                    v_sb[h][:, VSTRIDE * blk0:VSTRIDE * (blk0 + 4)]
                    .rearrange("a (n c) -> a n c", c=VSTRIDE)[:, :, 0:64]
                )
                nc.vector.tensor_copy(dst, src)
            yield

        def attn_pair_units(b, j4):
            """Attention for sq-chunk (b, j4): loop over sk tile-pairs.

            y matmuls are emitted one pair behind the scores/exp so the
            in-order PE queue never blocks on the Act-engine exp."""
            ni = 4 * j4 + 4
            col0 = S * b + 512 * j4
            yps = [
                ps_y.tile([128, 512], F32, name=f"y_{b}_{j4}_{h}", tag="yps")
                for h in range(HPC)
            ]
            n_pair = ni // 2

            def emit_y(pt2, pr):
                i0 = 2 * pr
                for h in range(HPC):
                    blk0 = N_SK * b + i0
                    if USE_DR:
                        vv = (
                            v_sb[h][:, VSTRIDE * blk0:VSTRIDE * (blk0 + 2)]
                            .rearrange("a (n c) -> a n c", c=VSTRIDE)[:, :, 0:65]
                        )
                        nc.tensor.matmul(
                            yps[h][0:65, :],
                            vv,
                            pt2[:, 1024 * h:1024 * (h + 1)]
                            .rearrange("a (n c) -> a n c", c=512),
                            start=(pr == 0),
                            stop=(pr == n_pair - 1),
                            perf_mode=mybir.MatmulPerfMode.DoubleRow,
                        )
                    else:
                        for jj in range(2):
                            blk = blk0 + jj
                            vv = v_sb[h][:, VSTRIDE * blk:VSTRIDE * blk + 65]
                            nc.tensor.matmul(
                                yps[h][0:65, :],
                                vv,
                                pt2[:, 1024 * h + 512 * jj:
                                    1024 * h + 512 * (jj + 1)],
                                start=(pr == 0 and jj == 0),
                                stop=(pr == n_pair - 1 and jj == 1),
                            )

            prev = None
            for pr in range(n_pair):
                i0 = 2 * pr
                # both heads' P tiles side by side: [h0 i0|i1, h1 i0|i1]
                pt2 = pt_pool.tile([128, 2048], PT_DT,
                                   name=f"pt_{b}_{j4}_{pr}", tag="ptt")
                for h in range(HPC):
                    sp = ps_sp.tile([128, 1024], F32,
                                    name=f"s_{b}_{j4}_{pr}_{h}", tag="sp")
                    for jj in range(2):
                        i = i0 + jj
                        nc.tensor.matmul(
                            sp[:, 512 * jj:512 * (jj + 1)],
                            qkvT[1][64 * h:64 * (h + 1),
                                    S * b + 128 * i:S * b + 128 * (i + 1)],
                            qkvT[0][64 * h:64 * (h + 1), col0:col0 + 512],
                            start=True, stop=True,
                        )
                    nc.scalar.activation(
                        pt2[:, 1024 * h:1024 * (h + 1)], sp,
                        mybir.ActivationFunctionType.Exp,
                        scale=0.125, bias=expbias[:, 0:1],
                    )
                if 128 * (i0 + 1) + 127 > 512 * j4:  # pair straddles diag
                    ptv = pt2.rearrange("a (h n c) -> a h n c", h=2, c=512)
                    nc.gpsimd.affine_select(
                        out=ptv, in_=ptv,
                        pattern=[[0, 2], [-128, 2], [1, 512]],
                        channel_multiplier=-1,
                        base=512 * j4 - 128 * i0,
                        compare_op=mybir.AluOpType.is_ge,
                        fill=0.0,
                    )
                if DEBUG and (b, j4, pr) == (0, 0, 0):
                    nc.sync.dma_start(out=dbg_pt, in_=pt2[:, 0:1024])
                if prev is not None:
                    emit_y(*prev)
                prev = (pt2, pr)
                yield
            emit_y(*prev)
            # softmax normalization: fast reciprocal of both heads' sums,
            # one partition_broadcast for both, then per-head multiply
            sums_sb = small_pool.tile([1, 1024], F32,
                                      name=f"sm_{b}_{j4}", tag="sm")
            for h in range(HPC):
                nc.vector.tensor_copy(
                    sums_sb[:, 512 * h:512 * (h + 1)], yps[h][64:65, :]
                )
            recip = small_pool.tile([1, 1024], F32,
                                    name=f"rc_{b}_{j4}", tag="rc")
            nc.vector.reciprocal_approx_fast(out=recip, in_=sums_sb)
            if DEBUG:
                idx = (b * N_CH + j4)
                nc.sync.dma_start(out=dbg_rc[2 * idx:2 * idx + 1, :],
                                  in_=recip[:, 0:512])
            bc = small_pool.tile([64, 1024], F32,
                                 name=f"bc_{b}_{j4}", tag="bc")
            nc.gpsimd.partition_broadcast(bc, recip)
            for h in range(HPC):
                nc.vector.tensor_mul(
                    ynorm[64 * h:64 * (h + 1), col0:col0 + 512],
                    yps[h][0:64, :],
                    bc[:, 512 * h:512 * (h + 1)],
                )
            yield

        def proj_units(b, j4):
            """Projection + output DMA for sq-chunk (b, j4)."""
            for t in range(4):
                r0 = S * b + 512 * j4 + 128 * t
                ob = ob_pool.tile([128, C], BF16, name=f"ob_{b}_{j4}_{t}", tag="ob")
                for n in range(2):
                    pp = ps_u.tile([128, 512], F32,
                                   name=f"pp_{b}_{j4}_{t}_{n}", tag="u512")
                    nc.tensor.matmul(
                        pp,
                        ynorm[:, r0:r0 + 128],
                        wproj_sb[:, 512 * n:512 * (n + 1)],
                        start=True, stop=True,
                    )
                    nc.vector.tensor_copy(ob[:, 512 * n:512 * (n + 1)], pp)
                nc.sync.dma_start(out=out[r0:r0 + 128, :], in_=ob)
                if t % 2 == 1:
                    yield

        # ---------- scheduler ----------
        prep_q = deque()
        for b in range(B):
            for j in range(N_CH):
                prep_q.append((b, j, prep_chunk_units(b, j)))
        prep_done = set()
        work_q = deque()  # proj generators

        def pump_prep():
            """Advance the frontmost prep generator by one unit."""
            if not prep_q:
                return False
            b, j, g = prep_q[0]
            try:
                next(g)
            except StopIteration:
                prep_done.add((b, j))
                prep_q.popleft()
            return True

        def pump_work():
            if not work_q:
                return False
            g = work_q[0]
            try:
                next(g)
            except StopIteration:
                work_q.popleft()
            return True

        def pump_background(n):
            # prefer prep; hold the most recent proj generator in reserve so
            # the final (longest) attention chunk has PE work to hide its exp
            for _ in range(n):
                if pump_prep():
                    continue
                if len(work_q) > 1:
                    pump_work()
                else:
                    return

        # first x chunk's DMAs go out before the weight DMAs
        first_prep = prep_chunk_units(0, 0)
        next(first_prep)
        load_weights()
        for h in range(HPC):
            nc.vector.memset(v_sb[h], 1.0)
        prep_q[0] = (0, 0, first_prep)

        for b in range(B):
            for j4 in range(N_CH):
                while (b, j4) not in prep_done:
                    pump_prep()
                ag = attn_pair_units(b, j4)
                for _ in ag:
                    pump_background(2)
                work_q.append(proj_units(b, j4))
                pump_background(1)
        while work_q or prep_q:
            if not pump_work():
                pump_prep()
        if DEBUG:
            nc.sync.dma_start(out=dbg_v, in_=v_sb[0])

    nc.compile()
    return nc


def shard_inputs(x, w_attn, b_attn, w_proj):
    """Build the 8 per-core input maps (bf16 weights/activations)."""
    import ml_dtypes

    bf16 = ml_dtypes.bfloat16
    xf = np.ascontiguousarray(
        np.asarray(x, dtype=np.float32).reshape(-1, C)
    ).astype(bf16)
    w_attn = np.asarray(w_attn, dtype=np.float32)
    b_attn = np.asarray(b_attn, dtype=np.float32)
    w_proj = np.asarray(w_proj, dtype=np.float32)
    in_maps = []
    for c in range(N_CORES):
        heads = [HPC * c + h for h in range(HPC)]
        cols = []
        for part in range(3):  # q, k, v
            for h in heads:
                cols.append(np.arange(part * C + D * h, part * C + D * (h + 1)))
        cols = np.concatenate(cols)
        w_qkv_c = np.ascontiguousarray(w_attn[:, cols]).astype(bf16)
        b_qkv_c = np.ascontiguousarray(b_attn[cols].reshape(-1, 1))
        w_proj_c = np.ascontiguousarray(
            w_proj[D * heads[0]:D * (heads[-1] + 1), :]
        ).astype(bf16)
        in_maps.append(
            {"x": xf, "w_qkv": w_qkv_c, "b_qkv": b_qkv_c, "w_proj": w_proj_c}
        )
    return in_maps


def kernel(x, w_attn, b_attn, w_proj, b_proj):
    global LAST_EXEC_NS, LAST_RES
    x = np.asarray(x, dtype=np.float32)
    Bv, Sv, Cv = x.shape
    assert (Bv, Sv, Cv) == (B, S, C), (Bv, Sv, Cv)
    nc = build_nc()
    in_maps = shard_inputs(x, w_attn, b_attn, w_proj)
    extra = {}
    if os.environ.get("ATTN_TMPDIR"):
        extra["tmpdir"] = os.environ["ATTN_TMPDIR"]
    res = run_bass_kernel_spmd(nc, in_maps, list(range(N_CORES)), **extra)
    LAST_EXEC_NS = res.exec_time_ns
    LAST_RES = res
    acc = np.zeros((B * S, C), dtype=np.float32)
    for r in res.results:
        acc += np.asarray(r["out"]).astype(np.float32)
    acc += np.asarray(b_proj, dtype=np.float32)[None, :]
    return acc.reshape(B, S, C)
